# revision 1
# baseline (speedup 1.0000x reference)
"""Trainium2 Bass kernel for the char-LSTM word-similarity CNN scorer.

Problem: B=8192 examples x NW=4 words x L=16 chars. Per word: char
embeddings -> masked LSTMCell over <=16 steps -> cell state c [128].
Per example: 4x4 cosine matrix of the word reps -> 2-layer 2x2-valid
CNN -> linear scorer -> sigmoid.

Strategy (pure data parallel, 1024 examples / 4096 words per core):
 - Host folds emb @ W_ih.T + (b_ih + b_hh) into a [65, 512] table G65;
   per-step char inputs become a K=65 one-hot matmul (row 64 = "freeze"
   flag that drives f->1, i->0 for words past their length, so no
   masking/select ops are needed on device).
 - Words are sorted by length (desc) on host; step t only processes
   ceil(max_core_Nt/512) blocks of 512 words. State lives as
   [H=128 partitions, word] so the recurrent matmul is weights-stationary
   with zero transposes.
 - Tail: transpose c to [word, H], l2-normalize rows via ln/exp,
   round-trip through DRAM with an indirect-DMA gather to undo the sort
   and group words per example, cosine = fused mul+reduce per word pair,
   then the 2x2 convs + scorer lowered to tiny host-built matmuls.
"""

import os
import sys

for _p in ("/opt/trn_rl_repo",):
    if _p not in sys.path and os.path.isdir(_p):
        sys.path.insert(0, _p)

import ml_dtypes
import numpy as np

import concourse.bass as bass
import concourse.mybir as mybir
import concourse.tile as tile
from concourse.bass_utils import run_bass_kernel_spmd
from concourse.masks import make_identity

# This container's walrus build rejects CTRL instructions (Drain) carrying
# more than 2 sync waits ("Too many sync wait commands" in setupSyncWait).
# Tile's kernel-tail drain accumulates one wait per engine/DMA-queue sem, so
# redistribute: keep one wait on the drain, move the rest onto nofuse NOPs
# that execute before the all-engine barrier. Semantics are unchanged (all
# waits still complete before the barrier / semaphore teardown).
def _patched_drain_and_barrier(self, tick_clock, wait_clock):
    nc = self.nc
    drain_inst = nc.sync.drain()
    wait_clock.add_sem_waits(
        drain_inst.ins, tile.ScopedClock({None: tick_clock.global_clock})
    )
    waits = list(drain_inst.ins.sync_info.on_wait)
    if len(waits) > 1:
        drain_inst.ins.sync_info.on_wait = waits[:1]
        for k in range(1, len(waits)):
            nop = nc.sync.nop(nofuse=True, hint="drain_wait_spill")
            if nop.ins.sync_info is None:
                nop.ins.sync_info = mybir.SyncInfo(on_wait=[], on_update=[])
            nop.ins.sync_info.on_wait = [waits[k]]
    nc.all_engine_barrier()
    assert self.sems is not None
    popped = nc._tile_sem_poison_stack.pop()
    assert popped is self._sem_poison
    nc.clear_and_free_semaphores(list(self.sems.allocated().values()))
    nc.all_engine_barrier()


tile.TileContext._drain_and_barrier = _patched_drain_and_barrier

def _spill_excess_waits(nc):
    """Walrus here rejects instructions with more than ~2 sync waits. Spill
    excess waits onto same-engine NoOps inserted just before the instruction
    (engines dispatch in program order, so waiting earlier on the same engine
    is equivalent)."""
    cnt = [0]
    for fn in nc.m.functions:
        for bb in fn.blocks:
            insts = list(bb.instructions)
            out = []
            changed = False
            for inst in insts:
                si = inst.sync_info
                waits = list(si.on_wait) if si is not None and si.on_wait else []
                max_waits = 1
                if len(waits) > max_waits:
                    changed = True
                    keep = waits[-max_waits:]
                    extra = waits[:-max_waits]
                    for j in range(0, len(extra), max_waits):
                        cnt[0] += 1
                        nop = mybir.InstNoOp(name=f"I-spillw-{cnt[0]}", ins=[], outs=[])
                        nop.engine = inst.engine
                        nop.sync_info = mybir.SyncInfo(
                            on_wait=extra[j:j + max_waits], on_update=[])
                        nop.bass_nofuse = True
                        nop.bass_priority = 0
                        nop.text_hint = "spillw"
                        nop.debug = inst.debug
                        out.append(nop)
                    si.on_wait = keep
                out.append(inst)
            if changed:
                bb.instructions = out

B, NW, L, E, H, V = 8192, 4, 16, 128, 128, 64
NCORES = 8
PER = B // NCORES          # 1024 examples per core
NWORD = PER * NW           # 4096 words per core
NBLK = NWORD // 512        # 8 blocks of 512 words
NEC = PER // 128           # 8 example-chunks of 128
BLK = 512
FB = 30.0                  # freeze bias magnitude
F32 = mybir.dt.float32
BF16 = mybir.dt.bfloat16
AF = mybir.ActivationFunctionType
ALU = mybir.AluOpType

P6 = [(0, 1), (0, 2), (0, 3), (1, 2), (1, 3), (2, 3)]


# ----------------------------------------------------------------- host prep

def _build_consts(inp):
    emb = np.asarray(inp["emb_i"], np.float32)
    W_ih = np.asarray(inp["W_ih"], np.float32)
    W_hh = np.asarray(inp["W_hh"], np.float32)
    b = np.asarray(inp["b_ih"], np.float32) + np.asarray(inp["b_hh"], np.float32)
    G65 = np.zeros((V + 2, 4 * H), np.float32)
    G65[:V] = emb @ W_ih.T + b
    G65[V, 0:H] = -FB
    G65[V, H:2 * H] = +FB
    WhhT = np.ascontiguousarray(W_hh.T)

    w1 = np.asarray(inp["conv1_w"], np.float32)
    b1 = np.asarray(inp["conv1_b"], np.float32)
    w2 = np.asarray(inp["conv2_w"], np.float32)
    b2 = np.asarray(inp["conv2_b"], np.float32)
    ws = np.asarray(inp["scorer_w"], np.float32)
    bs = float(np.asarray(inp["scorer_b"], np.float32)[0])

    p6idx = {p: i for i, p in enumerate(P6)}
    W1eff = np.zeros((6, 36), np.float32)
    b1eff = np.zeros((36, 1), np.float32)
    for c in range(4):
        for y in range(3):
            for x in range(3):
                m = c * 9 + y * 3 + x
                b1eff[m, 0] += b1[c]
                for dy in range(2):
                    for dx in range(2):
                        a, bb = y + dy, x + dx
                        w = w1[c, 0, dy, dx]
                        if a == bb:
                            b1eff[m, 0] += w
                        else:
                            W1eff[p6idx[(min(a, bb), max(a, bb))], m] += w
    W2eff = np.zeros((36, 32), np.float32)
    b2eff = np.zeros((32, 1), np.float32)
    for c2 in range(8):
        for y in range(2):
            for x in range(2):
                m = c2 * 4 + y * 2 + x
                b2eff[m, 0] = b2[c2]
                for c1 in range(4):
                    for dy in range(2):
                        for dx in range(2):
                            W2eff[c1 * 9 + (y + dy) * 3 + (x + dx), m] += w2[c2, c1, dy, dx]
    Wsc = ws[0].astype(np.float32).reshape(32, 1)
    return dict(G65=G65, WhhT=WhhT, W1eff=W1eff, b1eff=b1eff,
                W2eff=W2eff, b2eff=b2eff, Wsc=Wsc, bsc=bs)


def _core_prep(word_ids_c, lengths_c):
    wid = np.asarray(word_ids_c).reshape(NWORD, L)
    lens = np.asarray(lengths_c).reshape(NWORD)
    perm = np.argsort(-lens, kind="stable")
    inv = np.empty(NWORD, np.int32)
    inv[perm] = np.arange(NWORD, dtype=np.int32)
    wid_s = wid[perm]
    lens_s = lens[perm]
    Nt = (np.arange(L)[:, None] < lens_s[None, :]).sum(1)  # [L]
    return wid_s, lens_s, Nt, inv


def _build_onehot(wid_s, lens_s, widths):
    oh = np.zeros((L, V + 2, NWORD), np.float32)
    cols = np.arange(NWORD)
    for t in range(L):
        n = int(widths[t])
        if n == 0:
            continue
        alive = lens_s[:n] > t
        oh[t, wid_s[:n, t], cols[:n]] = alive.astype(np.float32)
        oh[t, V, cols[:n]] = (~alive).astype(np.float32)
    return oh


def _build_idx(inv):
    # idx[p, i*NEC + ec] = sorted-position of original word 4*(ec*128+p)+i
    idx = np.empty((128, NW * NEC), np.int32)
    p = np.arange(128)
    for i in range(NW):
        for ec in range(NEC):
            idx[:, i * NEC + ec] = inv[NW * (ec * 128 + p) + i]
    return idx


# -------------------------------------------------------------- bass program

def _build_program(bt):
    """bt: tuple of per-step block counts (len L, each 0..8)."""
    nc = bass.Bass()

    oh_in = nc.dram_tensor("oh", [L, V + 2, NWORD], BF16, kind="ExternalInput")
    idx_in = nc.dram_tensor("idx", [128, NW * NEC], mybir.dt.int32, kind="ExternalInput")
    g65_in = nc.dram_tensor("g65", [V + 2, 4 * H], BF16, kind="ExternalInput")
    whht_in = nc.dram_tensor("whht", [H, 4 * H], BF16, kind="ExternalInput")
    w1_in = nc.dram_tensor("w1", [6, 36], F32, kind="ExternalInput")
    b1_in = nc.dram_tensor("b1", [36, 1], F32, kind="ExternalInput")
    w2_in = nc.dram_tensor("w2", [36, 32], F32, kind="ExternalInput")
    b2_in = nc.dram_tensor("b2", [32, 1], F32, kind="ExternalInput")
    wsc_in = nc.dram_tensor("wsc", [32, 1], F32, kind="ExternalInput")
    bsc_in = nc.dram_tensor("bsc", [1, 1], F32, kind="ExternalInput")
    out_d = nc.dram_tensor("out", [1, PER], F32, kind="ExternalOutput")
    c_dram = nc.dram_tensor("cscratch", [NWORD, H], F32)

    with tile.TileContext(nc) as tc:
        with (
            tc.tile_pool(name="const", bufs=1) as cpool,
            tc.tile_pool(name="state", bufs=1) as spool,
        ):
            g65_sb = cpool.tile([V + 2, 4 * H], BF16, tag="g65", name="g65")
            whht_sb = cpool.tile([H, 4 * H], BF16, tag="whht", name="whht")
            w1_sb = cpool.tile([6, 36], F32, tag="w1", name="w1")
            b1_sb = cpool.tile([36, 1], F32, tag="b1", name="b1")
            w2_sb = cpool.tile([36, 32], F32, tag="w2", name="w2")
            b2_sb = cpool.tile([32, 1], F32, tag="b2", name="b2")
            wsc_sb = cpool.tile([32, 1], F32, tag="wsc", name="wsc")
            bsc_sb = cpool.tile([1, 1], F32, tag="bsc", name="bsc")
            ident = cpool.tile([128, 128], F32, tag="ident", name="ident")
            idx_sb = cpool.tile([128, NW * NEC], mybir.dt.int32, tag="idx", name="idx")
            for sb, dr in ((g65_sb, g65_in), (whht_sb, whht_in), (w1_sb, w1_in),
                           (b1_sb, b1_in), (w2_sb, w2_in), (b2_sb, b2_in),
                           (wsc_sb, wsc_in), (bsc_sb, bsc_in), (idx_sb, idx_in)):
                nc.sync.dma_start(sb[:], dr[:])
            make_identity(nc, ident[:])

            h_t = [spool.tile([H, BLK], BF16, tag=f"h{k}", name=f"h{k}") for k in range(NBLK)]
            c_t = [spool.tile([H, BLK], F32, tag=f"c{k}", name=f"c{k}") for k in range(NBLK)]
            for k in range(NBLK):
                nc.gpsimd.memset(c_t[k][:], 0.0)
                if bt[0] <= k:  # never matmul'd -> h must still be defined
                    nc.gpsimd.memset(h_t[k][:], 0.0)

            # ------------------------------------------------ LSTM main loop
            with (
                tc.tile_pool(name="oh", bufs=2) as ohpool,
                tc.tile_pool(name="gpsum", bufs=2, space="PSUM") as gpsum,
                tc.tile_pool(name="gsb", bufs=3) as gsb,
            ):
                for t in range(L):
                    nb = bt[t]
                    if nb == 0:
                        continue
                    w = nb * BLK
                    oh_sb = ohpool.tile([V + 2, NWORD], BF16, tag="oh", name="oh")
                    nc.sync.dma_start(oh_sb[:, :w], oh_in[t, :, :w])
                    last = t == L - 1
                    for k in range(nb):
                        ps = [gpsum.tile([128, BLK], F32, tag=f"p{m}", name=f"p{m}") for m in range(4)]
                        ohk = oh_sb[:, k * BLK:(k + 1) * BLK]
                        for m in range(4):
                            sl = slice(m * H, (m + 1) * H)
                            if t == 0:
                                nc.tensor.matmul(ps[m][:], lhsT=g65_sb[:, sl],
                                                 rhs=ohk, start=True, stop=True)
                            else:
                                nc.tensor.matmul(ps[m][:], lhsT=g65_sb[:, sl],
                                                 rhs=ohk, start=True, stop=False)
                                nc.tensor.matmul(ps[m][:], lhsT=whht_sb[:, sl],
                                                 rhs=h_t[k][:], start=False, stop=True)
                        ti = gsb.tile([128, BLK], F32, tag="ti", name="ti")
                        tf = gsb.tile([128, BLK], F32, tag="tf", name="tf")
                        tg = gsb.tile([128, BLK], F32, tag="tg", name="tg")
                        nc.scalar.activation(ti[:], ps[0][:], AF.Sigmoid)
                        nc.scalar.activation(tf[:], ps[1][:], AF.Sigmoid)
                        nc.scalar.activation(tg[:], ps[2][:], AF.Tanh)
                        nc.vector.tensor_mul(tg[:], ti[:], tg[:])      # i*g
                        nc.vector.tensor_mul(c_t[k][:], tf[:], c_t[k][:])
                        nc.vector.tensor_add(c_t[k][:], c_t[k][:], tg[:])
                        if not last:
                            to = gsb.tile([128, BLK], F32, tag="to", name="to")
                            tt = gsb.tile([128, BLK], F32, tag="tt", name="tt")
                            nc.scalar.activation(to[:], ps[3][:], AF.Sigmoid)
                            nc.scalar.activation(tt[:], c_t[k][:], AF.Tanh)
                            nc.vector.tensor_mul(h_t[k][:], to[:], tt[:])

            # ------------------------------------------------------- tail
            with (
                tc.tile_pool(name="big", bufs=1) as big,
                tc.tile_pool(name="tpsum", bufs=2, space="PSUM") as tpsum,
                tc.tile_pool(name="cpsum", bufs=2, space="PSUM") as cpsum,
                tc.tile_pool(name="small", bufs=1) as small,
                tc.tile_pool(name="scr", bufs=2) as scrp,
            ):
                cT = big.tile([128, NWORD], F32, tag="cT", name="cT")
                A = big.tile([128, NWORD], F32, tag="A", name="A")
                d_all = small.tile([128, 32], F32, tag="d", name="d")
                s_all = small.tile([128, 32], F32, tag="s", name="s")
                D_all = small.tile([128, NEC * 6], F32, tag="D", name="D")
                cos6 = small.tile([6, PER], F32, tag="cos6", name="cos6")

                for k in range(32):
                    pt = tpsum.tile([128, 128], F32, tag="tp", name="tp")
                    src = c_t[k // 4][:, (k % 4) * 128:(k % 4 + 1) * 128]
                    nc.tensor.transpose(pt[:], src, ident[:])
                    nc.vector.tensor_copy(cT[:, k * 128:(k + 1) * 128], pt[:])
                csq = big.tile([128, NWORD], F32, tag="csq", name="csq")
                nc.vector.tensor_mul(csq[:], cT[:], cT[:])
                nc.vector.tensor_reduce(
                    d_all[:], csq[:].rearrange("p (k h) -> p k h", k=32),
                    axis=mybir.AxisListType.X, op=ALU.add)
                nc.vector.tensor_scalar_max(d_all[:], d_all[:], 1e-30)
                nc.scalar.activation(s_all[:], d_all[:], AF.Ln)
                nc.scalar.activation(s_all[:], s_all[:], AF.Exp, scale=-0.5)
                for k in range(32):
                    cslc = cT[:, k * 128:(k + 1) * 128]
                    nc.vector.tensor_scalar_mul(cslc, cslc, s_all[:, k:k + 1])

                nc.sync.dma_start(
                    c_dram[:].rearrange("(k p) h -> p k h", p=128),
                    cT[:].rearrange("p (k h) -> p k h", k=32))

                for b in range(NW * NEC):
                    nc.gpsimd.indirect_dma_start(
                        out=A[:, b * 128:(b + 1) * 128],
                        out_offset=None,
                        in_=c_dram[:],
                        in_offset=bass.IndirectOffsetOnAxis(ap=idx_sb[:, b:b + 1], axis=0),
                    )

                Dp = small.tile([128, 6 * NEC], F32, tag="Dp", name="Dp")
                for k, (i, j) in enumerate(P6):
                    scr = scrp.tile([128, NEC * 128], F32, tag="scr", name="scr")
                    nc.vector.tensor_mul(
                        scr[:], A[:, i * NEC * 128:(i + 1) * NEC * 128],
                        A[:, j * NEC * 128:(j + 1) * NEC * 128])
                    nc.vector.tensor_reduce(
                        Dp[:, k * NEC:(k + 1) * NEC],
                        scr[:].rearrange("p (e h) -> p e h", e=NEC),
                        axis=mybir.AxisListType.X, op=ALU.add)
                for ec in range(NEC):
                    pt = tpsum.tile([128, 128], F32, tag="tp", name="tp")
                    dview = bass.AP(Dp.tensor, Dp.offset + ec,
                                    [Dp.ap[0], [NEC, 6]])
                    nc.tensor.transpose(pt[:6, :], dview, ident[:])
                    nc.vector.tensor_copy(cos6[:, ec * 128:(ec + 1) * 128], pt[:6, :])

                r1 = small.tile([36, PER], F32, tag="r1", name="r1")
                r2 = small.tile([32, PER], F32, tag="r2", name="r2")
                e_sb = small.tile([1, PER], F32, tag="e", name="e")
                o_sb = small.tile([1, PER], F32, tag="o", name="o")
                p1 = cpsum.tile([36, PER], F32, tag="cp1", name="cp1")
                for half in range(2):
                    sl = slice(half * 512, (half + 1) * 512)
                    nc.tensor.matmul(p1[:, sl], lhsT=w1_sb[:], rhs=cos6[:, sl],
                                     start=True, stop=True)
                nc.scalar.activation(r1[:], p1[:], AF.Relu, bias=b1_sb[:, 0:1])
                p2 = cpsum.tile([32, PER], F32, tag="cp1", name="cp1")
                for half in range(2):
                    sl = slice(half * 512, (half + 1) * 512)
                    nc.tensor.matmul(p2[:, sl], lhsT=w2_sb[:], rhs=r1[:, sl],
                                     start=True, stop=True)
                nc.scalar.activation(r2[:], p2[:], AF.Relu, bias=b2_sb[:, 0:1])
                p3 = cpsum.tile([1, PER], F32, tag="cp1", name="cp1")
                for half in range(2):
                    sl = slice(half * 512, (half + 1) * 512)
                    nc.tensor.matmul(p3[:, sl], lhsT=wsc_sb[:], rhs=r2[:, sl],
                                     start=True, stop=True)
                nc.scalar.activation(o_sb[:], p3[:], AF.Sigmoid,
                                     bias=bsc_sb[0:1, 0:1])
                nc.sync.dma_start(out_d[:], o_sb[:])

    return nc


_prog_cache = {}


def _get_program(bt):
    key = tuple(int(x) for x in bt)
    if key not in _prog_cache:
        _prog_cache[key] = _build_program(key)
    return _prog_cache[key]


def _run(inputs, trace=False):
    consts = _build_consts(inputs)
    word_ids = np.asarray(inputs["word_ids"])
    lengths = np.asarray(inputs["lengths"])

    preps = []
    for c in range(NCORES):
        sl = slice(c * PER, (c + 1) * PER)
        preps.append(_core_prep(word_ids[sl], lengths[sl]))
    Nt_max = np.stack([p[2] for p in preps]).max(0)
    bt = tuple(int(x) for x in np.ceil(Nt_max / BLK).astype(np.int64))
    widths = [b * BLK for b in bt]

    g65_bf = consts["G65"].astype(ml_dtypes.bfloat16)
    whht_bf = consts["WhhT"].astype(ml_dtypes.bfloat16)
    in_maps = []
    for c in range(NCORES):
        wid_s, lens_s, _, inv = preps[c]
        in_maps.append({
            "oh": _build_onehot(wid_s, lens_s, widths).astype(ml_dtypes.bfloat16),
            "idx": _build_idx(inv),
            "g65": g65_bf, "whht": whht_bf,
            "w1": consts["W1eff"], "b1": consts["b1eff"],
            "w2": consts["W2eff"], "b2": consts["b2eff"],
            "wsc": consts["Wsc"],
            "bsc": np.full((1, 1), consts["bsc"], np.float32),
        })

    nc = _get_program(bt)
    _spill_excess_waits(nc)  # idempotent; HW-compile only (CoreSim dislikes raw NoOps)
    res = run_bass_kernel_spmd(nc, in_maps, list(range(NCORES)), trace=trace)
    out = np.concatenate([np.asarray(r["out"]).reshape(PER) for r in res.results])
    return out.reshape(B, 1).astype(np.float32), res.exec_time_ns


def kernel(**inputs):
    return _run(inputs)[0]



# revision 2
# speedup vs baseline: 1.1672x; 1.1672x over previous
"""Trainium2 Bass kernel for the char-LSTM word-similarity CNN scorer.

Problem: B=8192 examples x NW=4 words x L=16 chars. Per word: char
embeddings -> masked LSTMCell over <=16 steps -> cell state c [128].
Per example: 4x4 cosine matrix of the word reps -> 2-layer 2x2-valid
CNN -> linear scorer -> sigmoid.

Strategy (pure data parallel, 1024 examples / 4096 words per core):
 - Host folds emb @ W_ih.T + (b_ih + b_hh) into a [66, 512] table G65
   with gate-column order (i, f, o, g); per-step char inputs become a
   K=66 one-hot matmul (row 64 = "freeze" flag driving f->1, i->0 for
   words past their length, so no masking/select ops on device).
 - Words sorted by length (desc) on host; step t processes exactly
   W[t] columns (max alive over cores, rounded to 16) in <=512-col
   chunks. Gate PSUM layout [i|f|o|g] at 512-col strides lets ONE wide
   Sigmoid ACTIVATE cover i,f,o (amortizing the ~352-cycle fixed cost);
   tanh(g)/tanh(c) are separate. sigma(o)/tanh(c)/h only computed on
   the next-step-alive prefix. Gate activations are bf16 so DVE
   tensor_tensor ops hit 2x mode where both operands are 16-bit.
 - Streamed tail: when a 128-col group of sorted words freezes
   (host-known step), cast c->bf16 (DVE copy), DMA-xbar transpose
   (idle DMA queue; no PSUM), indirect-scatter rows to DRAM in
   example-grouped order (idle GpSimd) - all hidden under the LSTM.
 - Post-loop: one strided readback -> A [128 ex-part, (word, h)],
   norms via square+reduce+ln/exp, 6 pair mul+reduce dots, rsqrt-norm
   scaling on the tiny [128, 8] dot tiles, PE transposes to [6, 1024],
   then the 2x2 convs + scorer as tiny host-built matmuls.
"""

import os
import sys

for _p in ("/opt/trn_rl_repo",):
    if _p not in sys.path and os.path.isdir(_p):
        sys.path.insert(0, _p)

import ml_dtypes
import numpy as np

import concourse.bass as bass
import concourse.mybir as mybir
import concourse.tile as tile
from concourse.bass_utils import run_bass_kernel_spmd
from concourse.masks import make_identity

# This container's walrus build rejects CTRL instructions (Drain) carrying
# more than 2 sync waits ("Too many sync wait commands" in setupSyncWait).
# Tile's kernel-tail drain accumulates one wait per engine/DMA-queue sem, so
# redistribute: keep one wait on the drain, move the rest onto nofuse NOPs
# that execute before the all-engine barrier. Semantics are unchanged (all
# waits still complete before the barrier / semaphore teardown).
def _patched_drain_and_barrier(self, tick_clock, wait_clock):
    nc = self.nc
    drain_inst = nc.sync.drain()
    wait_clock.add_sem_waits(
        drain_inst.ins, tile.ScopedClock({None: tick_clock.global_clock})
    )
    waits = list(drain_inst.ins.sync_info.on_wait)
    if len(waits) > 1:
        drain_inst.ins.sync_info.on_wait = waits[:1]
        for k in range(1, len(waits)):
            nop = nc.sync.nop(nofuse=True, hint="drain_wait_spill")
            if nop.ins.sync_info is None:
                nop.ins.sync_info = mybir.SyncInfo(on_wait=[], on_update=[])
            nop.ins.sync_info.on_wait = [waits[k]]
    nc.all_engine_barrier()
    assert self.sems is not None
    popped = nc._tile_sem_poison_stack.pop()
    assert popped is self._sem_poison
    nc.clear_and_free_semaphores(list(self.sems.allocated().values()))
    nc.all_engine_barrier()


tile.TileContext._drain_and_barrier = _patched_drain_and_barrier

def _spill_excess_waits(nc):
    """Walrus here rejects instructions with more than ~2 sync waits. Spill
    excess waits onto same-engine NoOps inserted just before the instruction
    (engines dispatch in program order, so waiting earlier on the same engine
    is equivalent)."""
    cnt = [0]
    for fn in nc.m.functions:
        for bb in fn.blocks:
            insts = list(bb.instructions)
            out = []
            changed = False
            for inst in insts:
                si = inst.sync_info
                waits = list(si.on_wait) if si is not None and si.on_wait else []
                max_waits = 1
                if len(waits) > max_waits:
                    changed = True
                    keep = waits[-max_waits:]
                    extra = waits[:-max_waits]
                    for j in range(0, len(extra), max_waits):
                        cnt[0] += 1
                        nop = mybir.InstNoOp(name=f"I-spillw-{cnt[0]}", ins=[], outs=[])
                        nop.engine = inst.engine
                        nop.sync_info = mybir.SyncInfo(
                            on_wait=extra[j:j + max_waits], on_update=[])
                        nop.bass_nofuse = True
                        nop.bass_priority = 0
                        nop.text_hint = "spillw"
                        nop.debug = inst.debug
                        out.append(nop)
                    si.on_wait = keep
                out.append(inst)
            if changed:
                bb.instructions = out

B, NW, L, E, H, V = 8192, 4, 16, 128, 128, 64
NCORES = 8
PER = B // NCORES          # 1024 examples per core
NWORD = PER * NW           # 4096 words per core
NEC = PER // 128           # 8 example-chunks of 128
BLK = 512                  # words per PSUM chunk
NG = NWORD // 128          # 32 groups of 128 sorted words
FB = 30.0                  # freeze bias magnitude
F32 = mybir.dt.float32
BF16 = mybir.dt.bfloat16
I32 = mybir.dt.int32
AF = mybir.ActivationFunctionType
ALU = mybir.AluOpType

P6 = [(0, 1), (0, 2), (0, 3), (1, 2), (1, 3), (2, 3)]


# ----------------------------------------------------------------- host prep

def _build_consts(inp):
    emb = np.asarray(inp["emb_i"], np.float32)
    W_ih = np.asarray(inp["W_ih"], np.float32)
    W_hh = np.asarray(inp["W_hh"], np.float32)
    b = np.asarray(inp["b_ih"], np.float32) + np.asarray(inp["b_hh"], np.float32)
    # gate-column reorder (torch i,f,g,o) -> (i,f,o,g)
    gorder = np.r_[0:H, H:2 * H, 3 * H:4 * H, 2 * H:3 * H]
    G = np.zeros((V + 2, 4 * H), np.float32)
    G[:V] = (emb @ W_ih.T + b)[:, gorder]
    G[V, 0:H] = -FB            # i -> 0
    G[V, H:2 * H] = +FB        # f -> 1
    WhhT = np.ascontiguousarray(W_hh.T[:, gorder])

    w1 = np.asarray(inp["conv1_w"], np.float32)
    b1 = np.asarray(inp["conv1_b"], np.float32)
    w2 = np.asarray(inp["conv2_w"], np.float32)
    b2 = np.asarray(inp["conv2_b"], np.float32)
    ws = np.asarray(inp["scorer_w"], np.float32)
    bs = float(np.asarray(inp["scorer_b"], np.float32)[0])

    p6idx = {p: i for i, p in enumerate(P6)}
    W1eff = np.zeros((6, 36), np.float32)
    b1eff = np.zeros((36, 1), np.float32)
    for c in range(4):
        for y in range(3):
            for x in range(3):
                m = c * 9 + y * 3 + x
                b1eff[m, 0] += b1[c]
                for dy in range(2):
                    for dx in range(2):
                        a, bb = y + dy, x + dx
                        w = w1[c, 0, dy, dx]
                        if a == bb:
                            b1eff[m, 0] += w
                        else:
                            W1eff[p6idx[(min(a, bb), max(a, bb))], m] += w
    W2eff = np.zeros((36, 32), np.float32)
    b2eff = np.zeros((32, 1), np.float32)
    for c2 in range(8):
        for y in range(2):
            for x in range(2):
                m = c2 * 4 + y * 2 + x
                b2eff[m, 0] = b2[c2]
                for c1 in range(4):
                    for dy in range(2):
                        for dx in range(2):
                            W2eff[c1 * 9 + (y + dy) * 3 + (x + dx), m] += w2[c2, c1, dy, dx]
    Wsc = ws[0].astype(np.float32).reshape(32, 1)
    return dict(G65=G, WhhT=WhhT, W1eff=W1eff, b1eff=b1eff,
                W2eff=W2eff, b2eff=b2eff, Wsc=Wsc, bsc=bs)


def _core_prep(word_ids_c, lengths_c):
    wid = np.asarray(word_ids_c).reshape(NWORD, L)
    lens = np.asarray(lengths_c).reshape(NWORD)
    perm = np.argsort(-lens, kind="stable")
    wid_s = wid[perm]
    lens_s = lens[perm]
    Nt = (np.arange(L)[:, None] < lens_s[None, :]).sum(1)  # alive count per step
    # scatter destination row (example-grouped layout) per sorted position
    e = perm // NW
    i = perm % NW
    dest = (i * PER + e).astype(np.int32)          # [NWORD]
    idx = np.ascontiguousarray(dest.reshape(NG, 128).T)  # [128, NG]
    return wid_s, lens_s, Nt, idx


def _build_onehot(wid_s, lens_s, W, off, tot):
    oh = np.zeros((V + 2, tot), np.float32)
    for t in range(L):
        n = int(W[t])
        if n == 0:
            continue
        ch = np.where(lens_s[:n] > t, wid_s[:n, t], V)
        oh[ch, off[t] + np.arange(n)] = 1.0
    return oh


# -------------------------------------------------------------- bass program

def _schedule(W):
    """W: per-step widths. Returns (off, tot, groups_by_t)."""
    off = np.zeros(L, np.int64)
    for t in range(1, L):
        off[t] = off[t - 1] + W[t - 1]
    tot = int(off[-1] + W[-1])
    # group g (cols [128g,128g+128)) finalizes after the last step with W > 128g
    groups_by_t = {t: [] for t in range(L)}
    for g in range(NG):
        fg = max(t for t in range(L) if W[t] > 128 * g)
        groups_by_t[fg].append(g)
    return off, tot, groups_by_t


def _build_program(W):
    W = list(W) + [0]
    off, tot, groups_by_t = _schedule(W[:L])

    nc = bass.Bass()
    oh_in = nc.dram_tensor("oh", [V + 2, tot], BF16, kind="ExternalInput")
    idx_in = nc.dram_tensor("idx", [128, NG], I32, kind="ExternalInput")
    g65_in = nc.dram_tensor("g65", [V + 2, 4 * H], BF16, kind="ExternalInput")
    whht_in = nc.dram_tensor("whht", [H, 4 * H], BF16, kind="ExternalInput")
    w1_in = nc.dram_tensor("w1", [6, 36], F32, kind="ExternalInput")
    b1_in = nc.dram_tensor("b1", [36, 1], F32, kind="ExternalInput")
    w2_in = nc.dram_tensor("w2", [36, 32], F32, kind="ExternalInput")
    b2_in = nc.dram_tensor("b2", [32, 1], F32, kind="ExternalInput")
    wsc_in = nc.dram_tensor("wsc", [32, 1], F32, kind="ExternalInput")
    bsc_in = nc.dram_tensor("bsc", [1, 1], F32, kind="ExternalInput")
    out_d = nc.dram_tensor("out", [1, PER], F32, kind="ExternalOutput")
    e_dram = nc.dram_tensor("escratch", [NWORD, H], BF16)

    with tile.TileContext(nc) as tc:
        with (
            tc.tile_pool(name="const", bufs=1) as cpool,
            tc.tile_pool(name="state", bufs=1) as spool,
        ):
            g65_sb = cpool.tile([V + 2, 4 * H], BF16, tag="g65", name="g65")
            whht_sb = cpool.tile([H, 4 * H], BF16, tag="whht", name="whht")
            idx_sb = cpool.tile([128, NG], I32, tag="idx", name="idx")
            w1_sb = cpool.tile([6, 36], F32, tag="w1", name="w1")
            b1_sb = cpool.tile([36, 1], F32, tag="b1", name="b1")
            w2_sb = cpool.tile([36, 32], F32, tag="w2", name="w2")
            b2_sb = cpool.tile([32, 1], F32, tag="b2", name="b2")
            wsc_sb = cpool.tile([32, 1], F32, tag="wsc", name="wsc")
            bsc_sb = cpool.tile([1, 1], F32, tag="bsc", name="bsc")
            ident = cpool.tile([128, 128], F32, tag="ident", name="ident")
            for sb, dr in ((g65_sb, g65_in), (whht_sb, whht_in), (idx_sb, idx_in),
                           (w1_sb, w1_in), (b1_sb, b1_in), (w2_sb, w2_in),
                           (b2_sb, b2_in), (wsc_sb, wsc_in), (bsc_sb, bsc_in)):
                nc.sync.dma_start(sb[:], dr[:])
            make_identity(nc, ident[:])

            NBLK = (max(W[:L]) + BLK - 1) // BLK
            c_blk = [spool.tile([H, BLK], F32, tag=f"c{k}", name=f"c{k}")
                     for k in range(NBLK)]
            h_blk = [spool.tile([H, BLK], BF16, tag=f"h{k}", name=f"h{k}")
                     for k in range(NBLK)]

            # ------------------------------------------------ LSTM main loop
            with (
                tc.tile_pool(name="oh", bufs=2) as ohpool,
                tc.tile_pool(name="gates", bufs=2, space="PSUM") as gpsum,
                tc.tile_pool(name="act", bufs=3) as apool,
                tc.tile_pool(name="scat", bufs=2) as scpool,
            ):
                for t in range(L):
                    Wt = W[t]
                    if Wt == 0:
                        continue
                    ct = (Wt + BLK - 1) // BLK
                    oh_sb = ohpool.tile([V + 2, NWORD], BF16, tag="oh", name="oh")
                    nc.sync.dma_start(oh_sb[:, :Wt], oh_in[:, int(off[t]):int(off[t]) + Wt])
                    for k in range(ct):
                        w = min(BLK, Wt - BLK * k)
                        wn = max(0, min(W[t + 1] - BLK * k, w))  # next-alive prefix
                        P = gpsum.tile([128, 4 * BLK], F32, tag="gates", name="gates")
                        for m in range(4):
                            nc.tensor.matmul(
                                P[:, BLK * m:BLK * m + w],
                                lhsT=g65_sb[:, H * m:H * (m + 1)],
                                rhs=oh_sb[:, BLK * k:BLK * k + w],
                                start=True, stop=(t == 0))
                        if t > 0:
                            for m in range(4):
                                nc.tensor.matmul(
                                    P[:, BLK * m:BLK * m + w],
                                    lhsT=whht_sb[:, H * m:H * (m + 1)],
                                    rhs=h_blk[k][:, :w],
                                    start=False, stop=True)
                        ifo = apool.tile([128, 3 * BLK], BF16, tag="ifo", name="ifo")
                        nc.scalar.activation(ifo[:, :2 * BLK + w], P[:, :2 * BLK + w],
                                             AF.Sigmoid)
                        gt = apool.tile([128, BLK], BF16, tag="g", name="g")
                        nc.scalar.activation(gt[:, :w], P[:, 3 * BLK:3 * BLK + w],
                                             AF.Tanh)
                        if t == 0:
                            nc.vector.tensor_mul(c_blk[k][:, :w],
                                                 ifo[:, :w], gt[:, :w])
                        else:
                            u = apool.tile([128, BLK], BF16, tag="u", name="u")
                            nc.vector.tensor_mul(u[:, :w], ifo[:, :w], gt[:, :w])
                            nc.vector.tensor_mul(c_blk[k][:, :w],
                                                 ifo[:, BLK:BLK + w],
                                                 c_blk[k][:, :w])
                            nc.vector.tensor_add(c_blk[k][:, :w],
                                                 c_blk[k][:, :w], u[:, :w])
                        if wn > 0:
                            tct = apool.tile([128, BLK], BF16, tag="tc", name="tc")
                            nc.scalar.activation(tct[:, :wn], c_blk[k][:, :wn],
                                                 AF.Tanh)
                            nc.vector.tensor_mul(h_blk[k][:, :wn],
                                                 ifo[:, 2 * BLK:2 * BLK + wn],
                                                 tct[:, :wn])
                    # finalize groups whose last alive step was t: cast ->
                    # xbar transpose -> indirect scatter (example layout)
                    for g in groups_by_t[t]:
                        blk, rel = g // 4, (g % 4) * 128
                        cb = scpool.tile([128, 128], BF16, tag="cb", name="cb")
                        nc.vector.tensor_copy(cb[:], c_blk[blk][:, rel:rel + 128])
                        tb = scpool.tile([128, 128], BF16, tag="tb", name="tb")
                        nc.sync.dma_start_transpose(tb[:], cb[:])
                        nc.gpsimd.indirect_dma_start(
                            out=e_dram[:],
                            out_offset=bass.IndirectOffsetOnAxis(
                                ap=idx_sb[:, g:g + 1], axis=0),
                            in_=tb[:],
                            in_offset=None,
                        )

            # ------------------------------------------------------- tail
            with (
                tc.tile_pool(name="big", bufs=1) as big,
                tc.tile_pool(name="tpsum", bufs=2, space="PSUM") as tpsum,
                tc.tile_pool(name="cpsum", bufs=2, space="PSUM") as cpsum,
                tc.tile_pool(name="small", bufs=1) as small,
                tc.tile_pool(name="scr", bufs=2) as scrp,
            ):
                A = big.tile([128, NWORD], BF16, tag="A", name="A")
                nc.sync.dma_start(
                    A[:].rearrange("p (b h) -> p b h", b=NW * NEC),
                    e_dram[:].rearrange("(b p) h -> p b h", p=128))
                Asq = big.tile([128, NWORD], BF16, tag="Asq", name="Asq")
                nc.vector.tensor_mul(Asq[:], A[:], A[:])
                d_all = small.tile([128, NW * NEC], F32, tag="d", name="d")
                nc.vector.tensor_reduce(
                    d_all[:], Asq[:].rearrange("p (b h) -> p b h", b=NW * NEC),
                    axis=mybir.AxisListType.X, op=ALU.add)
                nc.vector.tensor_scalar_max(d_all[:], d_all[:], 1e-30)
                rn = small.tile([128, NW * NEC], F32, tag="rn", name="rn")
                nc.scalar.activation(rn[:], d_all[:], AF.Ln)
                nc.scalar.activation(rn[:], rn[:], AF.Exp, scale=-0.5)

                Dp = small.tile([128, 6 * NEC], F32, tag="Dp", name="Dp")
                for k, (i, j) in enumerate(P6):
                    scr = scrp.tile([128, NEC * 128], BF16, tag="scr", name="scr")
                    nc.vector.tensor_mul(
                        scr[:], A[:, i * PER:(i + 1) * PER],
                        A[:, j * PER:(j + 1) * PER])
                    nc.vector.tensor_reduce(
                        Dp[:, k * NEC:(k + 1) * NEC],
                        scr[:].rearrange("p (e h) -> p e h", e=NEC),
                        axis=mybir.AxisListType.X, op=ALU.add)
                for k, (i, j) in enumerate(P6):
                    nc.vector.tensor_mul(Dp[:, k * NEC:(k + 1) * NEC],
                                         Dp[:, k * NEC:(k + 1) * NEC],
                                         rn[:, i * NEC:(i + 1) * NEC])
                    nc.vector.tensor_mul(Dp[:, k * NEC:(k + 1) * NEC],
                                         Dp[:, k * NEC:(k + 1) * NEC],
                                         rn[:, j * NEC:(j + 1) * NEC])

                cos6 = small.tile([6, PER], F32, tag="cos6", name="cos6")
                for ec in range(NEC):
                    pt = tpsum.tile([128, 128], F32, tag="tp", name="tp")
                    dview = bass.AP(Dp.tensor, Dp.offset + ec,
                                    [Dp.ap[0], [NEC, 6]])
                    nc.tensor.transpose(pt[:6, :], dview, ident[:])
                    nc.vector.tensor_copy(cos6[:, ec * 128:(ec + 1) * 128], pt[:6, :])

                r1 = small.tile([36, PER], F32, tag="r1", name="r1")
                r2 = small.tile([32, PER], F32, tag="r2", name="r2")
                o_sb = small.tile([1, PER], F32, tag="o", name="o")
                p1 = cpsum.tile([36, PER], F32, tag="cp1", name="cp1")
                for half in range(2):
                    sl = slice(half * 512, (half + 1) * 512)
                    nc.tensor.matmul(p1[:, sl], lhsT=w1_sb[:], rhs=cos6[:, sl],
                                     start=True, stop=True)
                nc.scalar.activation(r1[:], p1[:], AF.Relu, bias=b1_sb[:, 0:1])
                p2 = cpsum.tile([32, PER], F32, tag="cp1", name="cp1")
                for half in range(2):
                    sl = slice(half * 512, (half + 1) * 512)
                    nc.tensor.matmul(p2[:, sl], lhsT=w2_sb[:], rhs=r1[:, sl],
                                     start=True, stop=True)
                nc.scalar.activation(r2[:], p2[:], AF.Relu, bias=b2_sb[:, 0:1])
                p3 = cpsum.tile([1, PER], F32, tag="cp1", name="cp1")
                for half in range(2):
                    sl = slice(half * 512, (half + 1) * 512)
                    nc.tensor.matmul(p3[:, sl], lhsT=wsc_sb[:], rhs=r2[:, sl],
                                     start=True, stop=True)
                nc.scalar.activation(o_sb[:], p3[:], AF.Sigmoid,
                                     bias=bsc_sb[0:1, 0:1])
                nc.sync.dma_start(out_d[:], o_sb[:])

    return nc


_prog_cache = {}


def _get_program(W):
    key = tuple(int(x) for x in W)
    if key not in _prog_cache:
        _prog_cache[key] = _build_program(key)
    return _prog_cache[key]


def _run(inputs, trace=False):
    consts = _build_consts(inputs)
    word_ids = np.asarray(inputs["word_ids"])
    lengths = np.asarray(inputs["lengths"])

    preps = []
    for c in range(NCORES):
        sl = slice(c * PER, (c + 1) * PER)
        preps.append(_core_prep(word_ids[sl], lengths[sl]))
    Nt_max = np.stack([p[2] for p in preps]).max(0)
    W = tuple(int(min(NWORD, -(-int(n) // 16) * 16)) for n in Nt_max)
    off, tot, _ = _schedule(list(W))

    g65_bf = consts["G65"].astype(ml_dtypes.bfloat16)
    whht_bf = consts["WhhT"].astype(ml_dtypes.bfloat16)
    in_maps = []
    for c in range(NCORES):
        wid_s, lens_s, _, idx = preps[c]
        in_maps.append({
            "oh": _build_onehot(wid_s, lens_s, W, off, tot).astype(ml_dtypes.bfloat16),
            "idx": idx,
            "g65": g65_bf, "whht": whht_bf,
            "w1": consts["W1eff"], "b1": consts["b1eff"],
            "w2": consts["W2eff"], "b2": consts["b2eff"],
            "wsc": consts["Wsc"],
            "bsc": np.full((1, 1), consts["bsc"], np.float32),
        })

    nc = _get_program(W)
    _spill_excess_waits(nc)  # idempotent; HW-compile only
    res = run_bass_kernel_spmd(nc, in_maps, list(range(NCORES)), trace=trace)
    out = np.concatenate([np.asarray(r["out"]).reshape(PER) for r in res.results])
    return out.reshape(B, 1).astype(np.float32), res.exec_time_ns


def kernel(**inputs):
    return _run(inputs)[0]


# revision 6
# speedup vs baseline: 1.2211x; 1.0462x over previous
"""Trainium2 Bass kernel for the char-LSTM word-similarity CNN scorer.

Problem: B=8192 examples x NW=4 words x L=16 chars. Per word: char
embeddings -> masked LSTMCell over <=16 steps -> cell state c [128].
Per example: 4x4 cosine matrix of the word reps -> 2-layer 2x2-valid
CNN -> linear scorer -> sigmoid.

Strategy (pure data parallel, 1024 examples / 4096 words per core):
 - Host folds emb @ W_ih.T + (b_ih + b_hh) into a [66, 512] table G65
   with gate-column order (i, f, o, g); per-step char inputs become a
   K=66 one-hot matmul (row 64 = "freeze" flag driving f->1, i->0 for
   words past their length, so no masking/select ops on device).
 - Words sorted by length (desc) on host; step t processes exactly
   W[t] columns (max alive over cores, rounded to 16) in <=512-col
   chunks. Gate PSUM layout [i|f|o|g] at 512-col strides lets ONE wide
   Sigmoid ACTIVATE cover i,f,o (amortizing the ~352-cycle fixed cost);
   tanh(g)/tanh(c) are separate. sigma(o)/tanh(c)/h only computed on
   the next-step-alive prefix. Gate activations are bf16 so DVE
   tensor_tensor ops hit 2x mode where both operands are 16-bit.
 - Streamed tail: when a 128-col group of sorted words freezes
   (host-known step), cast c->bf16 (DVE copy), DMA-xbar transpose
   (idle DMA queue; no PSUM), indirect-scatter rows to DRAM in
   example-grouped order (idle GpSimd) - all hidden under the LSTM.
 - Post-loop: one strided readback -> A [128 ex-part, (word, h)],
   norms via square+reduce+ln/exp, 6 pair mul+reduce dots, rsqrt-norm
   scaling on the tiny [128, 8] dot tiles, PE transposes to [6, 1024],
   then the 2x2 convs + scorer as tiny host-built matmuls.
"""

import os
import sys

for _p in ("/opt/trn_rl_repo",):
    if _p not in sys.path and os.path.isdir(_p):
        sys.path.insert(0, _p)

import ml_dtypes
import numpy as np

import concourse.bass as bass
import concourse.mybir as mybir
import concourse.tile as tile
from concourse.bass_utils import run_bass_kernel_spmd
from concourse.masks import make_identity

# This container's walrus build rejects CTRL instructions (Drain) carrying
# more than 2 sync waits ("Too many sync wait commands" in setupSyncWait).
# Tile's kernel-tail drain accumulates one wait per engine/DMA-queue sem, so
# redistribute: keep one wait on the drain, move the rest onto nofuse NOPs
# that execute before the all-engine barrier. Semantics are unchanged (all
# waits still complete before the barrier / semaphore teardown).
def _patched_drain_and_barrier(self, tick_clock, wait_clock):
    nc = self.nc
    drain_inst = nc.sync.drain()
    wait_clock.add_sem_waits(
        drain_inst.ins, tile.ScopedClock({None: tick_clock.global_clock})
    )
    waits = list(drain_inst.ins.sync_info.on_wait)
    if len(waits) > 1:
        drain_inst.ins.sync_info.on_wait = waits[:1]
        for k in range(1, len(waits)):
            nop = nc.sync.nop(nofuse=True, hint="drain_wait_spill")
            if nop.ins.sync_info is None:
                nop.ins.sync_info = mybir.SyncInfo(on_wait=[], on_update=[])
            nop.ins.sync_info.on_wait = [waits[k]]
    nc.all_engine_barrier()
    assert self.sems is not None
    popped = nc._tile_sem_poison_stack.pop()
    assert popped is self._sem_poison
    nc.clear_and_free_semaphores(list(self.sems.allocated().values()))
    nc.all_engine_barrier()


tile.TileContext._drain_and_barrier = _patched_drain_and_barrier

def _spill_excess_waits(nc):
    """Walrus here rejects instructions with more than ~2 sync waits. Spill
    excess waits onto same-engine NoOps inserted just before the instruction
    (engines dispatch in program order, so waiting earlier on the same engine
    is equivalent)."""
    cnt = [0]
    for fn in nc.m.functions:
        for bb in fn.blocks:
            insts = list(bb.instructions)
            out = []
            changed = False
            for inst in insts:
                si = inst.sync_info
                waits = list(si.on_wait) if si is not None and si.on_wait else []
                max_waits = 1
                if len(waits) > max_waits:
                    changed = True
                    keep = waits[-max_waits:]
                    extra = waits[:-max_waits]
                    for j in range(0, len(extra), max_waits):
                        cnt[0] += 1
                        nop = mybir.InstNoOp(name=f"I-spillw-{cnt[0]}", ins=[], outs=[])
                        nop.engine = inst.engine
                        nop.sync_info = mybir.SyncInfo(
                            on_wait=extra[j:j + max_waits], on_update=[])
                        nop.bass_nofuse = True
                        nop.bass_priority = 0
                        nop.text_hint = "spillw"
                        nop.debug = inst.debug
                        out.append(nop)
                    si.on_wait = keep
                out.append(inst)
            if changed:
                bb.instructions = out

B, NW, L, E, H, V = 8192, 4, 16, 128, 128, 64
NCORES = 8
PER = B // NCORES          # 1024 examples per core
NWORD = PER * NW           # 4096 words per core
NEC = PER // 128           # 8 example-chunks of 128
BLK = 512                  # words per PSUM chunk
NG = NWORD // 128          # 32 groups of 128 sorted words
FB = 30.0                  # freeze bias magnitude
F32 = mybir.dt.float32
BF16 = mybir.dt.bfloat16
I32 = mybir.dt.int32
AF = mybir.ActivationFunctionType
ALU = mybir.AluOpType

P6 = [(0, 1), (0, 2), (0, 3), (1, 2), (1, 3), (2, 3)]


# ----------------------------------------------------------------- host prep

def _build_consts(inp):
    emb = np.asarray(inp["emb_i"], np.float32)
    W_ih = np.asarray(inp["W_ih"], np.float32)
    W_hh = np.asarray(inp["W_hh"], np.float32)
    b = np.asarray(inp["b_ih"], np.float32) + np.asarray(inp["b_hh"], np.float32)
    # gate-column reorder (torch i,f,g,o) -> (i,f,o,g)
    gorder = np.r_[0:H, H:2 * H, 3 * H:4 * H, 2 * H:3 * H]
    G = np.zeros((V + 2, 4 * H), np.float32)
    G[:V] = (emb @ W_ih.T + b)[:, gorder]
    G[V, 0:H] = -FB            # i -> 0
    G[V, H:2 * H] = +FB        # f -> 1
    WhhT = np.ascontiguousarray(W_hh.T[:, gorder])

    w1 = np.asarray(inp["conv1_w"], np.float32)
    b1 = np.asarray(inp["conv1_b"], np.float32)
    w2 = np.asarray(inp["conv2_w"], np.float32)
    b2 = np.asarray(inp["conv2_b"], np.float32)
    ws = np.asarray(inp["scorer_w"], np.float32)
    bs = float(np.asarray(inp["scorer_b"], np.float32)[0])

    p6idx = {p: i for i, p in enumerate(P6)}
    W1eff = np.zeros((6, 36), np.float32)
    b1eff = np.zeros((36, 1), np.float32)
    for c in range(4):
        for y in range(3):
            for x in range(3):
                m = c * 9 + y * 3 + x
                b1eff[m, 0] += b1[c]
                for dy in range(2):
                    for dx in range(2):
                        a, bb = y + dy, x + dx
                        w = w1[c, 0, dy, dx]
                        if a == bb:
                            b1eff[m, 0] += w
                        else:
                            W1eff[p6idx[(min(a, bb), max(a, bb))], m] += w
    W2eff = np.zeros((36, 32), np.float32)
    b2eff = np.zeros((32, 1), np.float32)
    for c2 in range(8):
        for y in range(2):
            for x in range(2):
                m = c2 * 4 + y * 2 + x
                b2eff[m, 0] = b2[c2]
                for c1 in range(4):
                    for dy in range(2):
                        for dx in range(2):
                            W2eff[c1 * 9 + (y + dy) * 3 + (x + dx), m] += w2[c2, c1, dy, dx]
    Wsc = ws[0].astype(np.float32).reshape(32, 1)
    return dict(G65=G, WhhT=WhhT, W1eff=W1eff, b1eff=b1eff,
                W2eff=W2eff, b2eff=b2eff, Wsc=Wsc, bsc=bs)


def _core_prep(word_ids_c, lengths_c):
    wid = np.asarray(word_ids_c).reshape(NWORD, L)
    lens = np.asarray(lengths_c).reshape(NWORD)
    perm = np.argsort(-lens, kind="stable")
    wid_s = wid[perm]
    lens_s = lens[perm]
    Nt = (np.arange(L)[:, None] < lens_s[None, :]).sum(1)  # alive count per step
    # scatter destination row (example-grouped layout) per sorted position
    e = perm // NW
    i = perm % NW
    dest = (i * PER + e).astype(np.int32)          # [NWORD]
    idx = np.ascontiguousarray(dest.reshape(NG, 128).T)  # [128, NG]
    return wid_s, lens_s, Nt, idx


def _build_onehot(wid_s, lens_s, W, off, tot):
    oh = np.zeros((V + 2, tot), np.float32)
    for t in range(L):
        n = int(W[t])
        if n == 0:
            continue
        ch = np.where(lens_s[:n] > t, wid_s[:n, t], V)
        oh[ch, off[t] + np.arange(n)] = 1.0
    return oh


# -------------------------------------------------------------- bass program

def _schedule(W):
    """W: per-step widths. Returns (off, tot, groups_by_t)."""
    off = np.zeros(L, np.int64)
    for t in range(1, L):
        off[t] = off[t - 1] + W[t - 1]
    tot = int(off[-1] + W[-1])
    # group g (cols [128g,128g+128)) finalizes after the last step with W > 128g
    groups_by_t = {t: [] for t in range(L)}
    for g in range(NG):
        fg = max(t for t in range(L) if W[t] > 128 * g)
        groups_by_t[fg].append(g)
    return off, tot, groups_by_t


def _build_program(W):
    W = list(W) + [0]
    off, tot, groups_by_t = _schedule(W[:L])

    nc = bass.Bass()
    oh_in = nc.dram_tensor("oh", [V + 2, tot], BF16, kind="ExternalInput")
    idx_in = nc.dram_tensor("idx", [128, NG], I32, kind="ExternalInput")
    g65_in = nc.dram_tensor("g65", [V + 2, 4 * H], BF16, kind="ExternalInput")
    whht_in = nc.dram_tensor("whht", [H, 4 * H], BF16, kind="ExternalInput")
    w1_in = nc.dram_tensor("w1", [6, 36], BF16, kind="ExternalInput")
    b1_in = nc.dram_tensor("b1", [36, 1], F32, kind="ExternalInput")
    w2_in = nc.dram_tensor("w2", [36, 32], BF16, kind="ExternalInput")
    b2_in = nc.dram_tensor("b2", [32, 1], F32, kind="ExternalInput")
    wsc_in = nc.dram_tensor("wsc", [32, 1], BF16, kind="ExternalInput")
    bsc_in = nc.dram_tensor("bsc", [1, 1], F32, kind="ExternalInput")
    out_d = nc.dram_tensor("out", [1, PER], F32, kind="ExternalOutput")
    e_dram = nc.dram_tensor("escratch", [NWORD, H], BF16)

    with tile.TileContext(nc) as tc:
        with (
            tc.tile_pool(name="const", bufs=1) as cpool,
            tc.tile_pool(name="state", bufs=1) as spool,
        ):
            g65_sb = cpool.tile([V + 2, 4 * H], BF16, tag="g65", name="g65")
            whht_sb = cpool.tile([H, 4 * H], BF16, tag="whht", name="whht")
            idx_sb = cpool.tile([128, NG], I32, tag="idx", name="idx")
            w1_sb = cpool.tile([6, 36], BF16, tag="w1", name="w1")
            b1_sb = cpool.tile([36, 1], F32, tag="b1", name="b1")
            w2_sb = cpool.tile([36, 32], BF16, tag="w2", name="w2")
            b2_sb = cpool.tile([32, 1], F32, tag="b2", name="b2")
            wsc_sb = cpool.tile([32, 1], BF16, tag="wsc", name="wsc")
            bsc_sb = cpool.tile([1, 1], F32, tag="bsc", name="bsc")
            ident = cpool.tile([128, 128], F32, tag="ident", name="ident")
            for sb, dr in ((g65_sb, g65_in), (whht_sb, whht_in), (idx_sb, idx_in),
                           (w1_sb, w1_in), (b1_sb, b1_in), (w2_sb, w2_in),
                           (b2_sb, b2_in), (wsc_sb, wsc_in), (bsc_sb, bsc_in)):
                nc.sync.dma_start(sb[:], dr[:])
            make_identity(nc, ident[:])

            NBLK = (max(W[:L]) + BLK - 1) // BLK
            c_blk = [spool.tile([H, BLK], F32, tag=f"c{k}", name=f"c{k}")
                     for k in range(NBLK)]
            h_blk = [spool.tile([H, BLK], BF16, tag=f"h{k}", name=f"h{k}")
                     for k in range(NBLK)]

            # ------------------------------------------------ LSTM main loop
            with (
                tc.tile_pool(name="oh", bufs=2) as ohpool,
                tc.tile_pool(name="gates", bufs=2, space="PSUM") as gpsum,
                tc.tile_pool(name="act", bufs=3) as apool,
                tc.tile_pool(name="scat", bufs=2) as scpool,
            ):
                oh_tiles = {}
                def load_oh(t):
                    if t >= L or W[t] == 0 or t in oh_tiles:
                        return
                    sb = ohpool.tile([V + 2, NWORD], BF16, tag="oh", name="oh")
                    nc.sync.dma_start(sb[:, :W[t]],
                                      oh_in[:, int(off[t]):int(off[t]) + W[t]])
                    oh_tiles[t] = sb

                load_oh(0)
                for t in range(L):
                    Wt = W[t]
                    if Wt == 0:
                        continue
                    ct = (Wt + BLK - 1) // BLK
                    oh_sb = oh_tiles.pop(t)
                    for k in range(ct):
                        w = min(BLK, Wt - BLK * k)
                        wn = max(0, min(W[t + 1] - BLK * k, w))  # next-alive prefix
                        P = gpsum.tile([128, 4 * BLK], F32, tag="gates", name="gates")
                        # sigma gates (i,f,o) first so the wide sigmoid can
                        # start before the g matmuls finish
                        for m in (0, 1, 2):
                            nc.tensor.matmul(
                                P[:, BLK * m:BLK * m + w],
                                lhsT=g65_sb[:, H * m:H * (m + 1)],
                                rhs=oh_sb[:, BLK * k:BLK * k + w],
                                start=True, stop=(t == 0))
                        if t > 0:
                            for m in (0, 1, 2):
                                nc.tensor.matmul(
                                    P[:, BLK * m:BLK * m + w],
                                    lhsT=whht_sb[:, H * m:H * (m + 1)],
                                    rhs=h_blk[k][:, :w],
                                    start=False, stop=True)
                        nc.tensor.matmul(
                            P[:, 3 * BLK:3 * BLK + w],
                            lhsT=g65_sb[:, 3 * H:4 * H],
                            rhs=oh_sb[:, BLK * k:BLK * k + w],
                            start=True, stop=(t == 0))
                        if t > 0:
                            nc.tensor.matmul(
                                P[:, 3 * BLK:3 * BLK + w],
                                lhsT=whht_sb[:, 3 * H:4 * H],
                                rhs=h_blk[k][:, :w],
                                start=False, stop=True)
                        if k == 0:
                            load_oh(t + 1)  # prefetch ahead of freeze-group DMAs
                        ifo = apool.tile([128, 3 * BLK], BF16, tag="ifo", name="ifo")
                        nc.scalar.activation(ifo[:, :2 * BLK + w], P[:, :2 * BLK + w],
                                             AF.Sigmoid)
                        gt = apool.tile([128, BLK], BF16, tag="g", name="g")
                        nc.scalar.activation(gt[:, :w], P[:, 3 * BLK:3 * BLK + w],
                                             AF.Tanh)
                        if t == 0:
                            nc.vector.tensor_mul(c_blk[k][:, :w],
                                                 ifo[:, :w], gt[:, :w])
                        else:
                            u = apool.tile([128, BLK], BF16, tag="u", name="u")
                            nc.vector.tensor_mul(u[:, :w], ifo[:, :w], gt[:, :w])
                            nc.vector.tensor_mul(c_blk[k][:, :w],
                                                 ifo[:, BLK:BLK + w],
                                                 c_blk[k][:, :w])
                            nc.vector.tensor_add(c_blk[k][:, :w],
                                                 c_blk[k][:, :w], u[:, :w])
                        if wn > 0:
                            tct = apool.tile([128, BLK], BF16, tag="tc", name="tc")
                            nc.scalar.activation(tct[:, :wn], c_blk[k][:, :wn],
                                                 AF.Tanh)
                            nc.vector.tensor_mul(h_blk[k][:, :wn],
                                                 ifo[:, 2 * BLK:2 * BLK + wn],
                                                 tct[:, :wn])
                    # finalize groups whose last alive step was t: cast ->
                    # xbar transpose -> indirect scatter (example layout)
                    for g in groups_by_t[t]:
                        blk, rel = g // 4, (g % 4) * 128
                        cb = scpool.tile([128, 128], BF16, tag="cb", name="cb")
                        nc.vector.tensor_copy(cb[:], c_blk[blk][:, rel:rel + 128])
                        tb = scpool.tile([128, 128], BF16, tag="tb", name="tb")
                        nc.sync.dma_start_transpose(tb[:], cb[:])
                        nc.gpsimd.indirect_dma_start(
                            out=e_dram[:],
                            out_offset=bass.IndirectOffsetOnAxis(
                                ap=idx_sb[:, g:g + 1], axis=0),
                            in_=tb[:],
                            in_offset=None,
                        )

            # ------------------------------------------------------- tail
            with (
                tc.tile_pool(name="big", bufs=1) as big,
                tc.tile_pool(name="tpsum", bufs=2, space="PSUM") as tpsum,
                tc.tile_pool(name="cpsum", bufs=2, space="PSUM") as cpsum,
                tc.tile_pool(name="small", bufs=1) as small,
                tc.tile_pool(name="scr", bufs=2) as scrp,
            ):
                A = big.tile([128, NWORD], BF16, tag="A", name="A")
                NB4 = NW * NEC // 4
                for q in range(4):
                    nc.sync.dma_start(
                        A[:, q * NB4 * H:(q + 1) * NB4 * H].rearrange(
                            "p (b h) -> p b h", b=NB4),
                        e_dram[q * NB4 * 128:(q + 1) * NB4 * 128, :].rearrange(
                            "(b p) h -> p b h", p=128))
                Asq = big.tile([128, NWORD], BF16, tag="Asq", name="Asq")
                nc.vector.tensor_mul(Asq[:], A[:], A[:])
                d_all = small.tile([128, NW * NEC], F32, tag="d", name="d")
                nc.vector.tensor_reduce(
                    d_all[:], Asq[:].rearrange("p (b h) -> p b h", b=NW * NEC),
                    axis=mybir.AxisListType.X, op=ALU.add)
                nc.vector.tensor_scalar_max(d_all[:], d_all[:], 1e-30)
                rn = small.tile([128, NW * NEC], F32, tag="rn", name="rn")
                nc.scalar.activation(rn[:], d_all[:], AF.Ln)
                nc.scalar.activation(rn[:], rn[:], AF.Exp, scale=-0.5)

                Dp = small.tile([128, 6 * NEC], F32, tag="Dp", name="Dp")
                for k, (i, j) in enumerate(P6):
                    scr = scrp.tile([128, NEC * 128], BF16, tag="scr", name="scr")
                    nc.vector.tensor_mul(
                        scr[:], A[:, i * PER:(i + 1) * PER],
                        A[:, j * PER:(j + 1) * PER])
                    nc.vector.tensor_reduce(
                        Dp[:, k * NEC:(k + 1) * NEC],
                        scr[:].rearrange("p (e h) -> p e h", e=NEC),
                        axis=mybir.AxisListType.X, op=ALU.add)
                for k, (i, j) in enumerate(P6):
                    nc.vector.tensor_mul(Dp[:, k * NEC:(k + 1) * NEC],
                                         Dp[:, k * NEC:(k + 1) * NEC],
                                         rn[:, i * NEC:(i + 1) * NEC])
                    nc.vector.tensor_mul(Dp[:, k * NEC:(k + 1) * NEC],
                                         Dp[:, k * NEC:(k + 1) * NEC],
                                         rn[:, j * NEC:(j + 1) * NEC])

                cos6 = small.tile([6, PER], BF16, tag="cos6", name="cos6")
                for ec in range(NEC):
                    pt = tpsum.tile([128, 128], F32, tag="tp", name="tp")
                    dview = bass.AP(Dp.tensor, Dp.offset + ec,
                                    [Dp.ap[0], [NEC, 6]])
                    nc.tensor.transpose(pt[:6, :], dview, ident[:])
                    nc.vector.tensor_copy(cos6[:, ec * 128:(ec + 1) * 128], pt[:6, :])

                r1 = small.tile([36, PER], BF16, tag="r1", name="r1")
                r2 = small.tile([32, PER], BF16, tag="r2", name="r2")
                o_sb = small.tile([1, PER], F32, tag="o", name="o")
                p1 = cpsum.tile([36, PER], F32, tag="cp1", name="cp1")
                for half in range(2):
                    sl = slice(half * 512, (half + 1) * 512)
                    nc.tensor.matmul(p1[:, sl], lhsT=w1_sb[:], rhs=cos6[:, sl],
                                     start=True, stop=True)
                nc.scalar.activation(r1[:], p1[:], AF.Relu, bias=b1_sb[:, 0:1])
                p2 = cpsum.tile([32, PER], F32, tag="cp1", name="cp1")
                for half in range(2):
                    sl = slice(half * 512, (half + 1) * 512)
                    nc.tensor.matmul(p2[:, sl], lhsT=w2_sb[:], rhs=r1[:, sl],
                                     start=True, stop=True)
                nc.scalar.activation(r2[:], p2[:], AF.Relu, bias=b2_sb[:, 0:1])
                p3 = cpsum.tile([1, PER], F32, tag="cp1", name="cp1")
                for half in range(2):
                    sl = slice(half * 512, (half + 1) * 512)
                    nc.tensor.matmul(p3[:, sl], lhsT=wsc_sb[:], rhs=r2[:, sl],
                                     start=True, stop=True)
                nc.scalar.activation(o_sb[:], p3[:], AF.Sigmoid,
                                     bias=bsc_sb[0:1, 0:1])
                nc.sync.dma_start(out_d[:], o_sb[:])

    return nc


_prog_cache = {}


def _get_program(W):
    key = tuple(int(x) for x in W)
    if key not in _prog_cache:
        _prog_cache[key] = _build_program(key)
    return _prog_cache[key]


def _run(inputs, trace=False):
    consts = _build_consts(inputs)
    word_ids = np.asarray(inputs["word_ids"])
    lengths = np.asarray(inputs["lengths"])

    preps = []
    for c in range(NCORES):
        sl = slice(c * PER, (c + 1) * PER)
        preps.append(_core_prep(word_ids[sl], lengths[sl]))
    Nt_max = np.stack([p[2] for p in preps]).max(0)
    W = tuple(int(min(NWORD, -(-int(n) // 16) * 16)) for n in Nt_max)
    off, tot, _ = _schedule(list(W))

    g65_bf = consts["G65"].astype(ml_dtypes.bfloat16)
    whht_bf = consts["WhhT"].astype(ml_dtypes.bfloat16)
    in_maps = []
    for c in range(NCORES):
        wid_s, lens_s, _, idx = preps[c]
        in_maps.append({
            "oh": _build_onehot(wid_s, lens_s, W, off, tot).astype(ml_dtypes.bfloat16),
            "idx": idx,
            "g65": g65_bf, "whht": whht_bf,
            "w1": consts["W1eff"].astype(ml_dtypes.bfloat16), "b1": consts["b1eff"],
            "w2": consts["W2eff"].astype(ml_dtypes.bfloat16), "b2": consts["b2eff"],
            "wsc": consts["Wsc"].astype(ml_dtypes.bfloat16),
            "bsc": np.full((1, 1), consts["bsc"], np.float32),
        })

    nc = _get_program(W)
    _spill_excess_waits(nc)  # idempotent; HW-compile only
    res = run_bass_kernel_spmd(nc, in_maps, list(range(NCORES)), trace=trace)
    out = np.concatenate([np.asarray(r["out"]).reshape(PER) for r in res.results])
    return out.reshape(B, 1).astype(np.float32), res.exec_time_ns


def kernel(**inputs):
    return _run(inputs)[0]


# revision 11
# speedup vs baseline: 1.2435x; 1.0184x over previous
"""Trainium2 Bass kernel for the char-LSTM word-similarity CNN scorer.

Problem: B=8192 examples x NW=4 words x L=16 chars. Per word: char
embeddings -> masked LSTMCell over <=16 steps -> cell state c [128].
Per example: 4x4 cosine matrix of the word reps -> 2-layer 2x2-valid
CNN -> linear scorer -> sigmoid.

Strategy (pure data parallel, 1024 examples / 4096 words per core):
 - Host folds emb @ W_ih.T + (b_ih + b_hh) into a [66, 512] table G65
   with gate-column order (i, f, o, g); per-step char inputs become a
   K=66 one-hot matmul (row 64 = "freeze" flag driving f->1, i->0 for
   words past their length, so no masking/select ops on device).
 - Words sorted by length (desc) on host; step t processes exactly
   W[t] columns (max alive over cores, rounded to 16) in <=512-col
   chunks. Gate PSUM layout [i|f|o|g] at 512-col strides lets ONE wide
   Sigmoid ACTIVATE cover i,f,o (amortizing the ~352-cycle fixed cost);
   tanh(g)/tanh(c) are separate. sigma(o)/tanh(c)/h only computed on
   the next-step-alive prefix. Gate activations are bf16 so DVE
   tensor_tensor ops hit 2x mode where both operands are 16-bit.
 - Streamed tail: when a 128-col group of sorted words freezes
   (host-known step), cast c->bf16 (DVE copy), DMA-xbar transpose
   (idle DMA queue; no PSUM), indirect-scatter rows to DRAM in
   example-grouped order (idle GpSimd) - all hidden under the LSTM.
 - Post-loop: one strided readback -> A [128 ex-part, (word, h)],
   norms via square+reduce+ln/exp, 6 pair mul+reduce dots, rsqrt-norm
   scaling on the tiny [128, 8] dot tiles, PE transposes to [6, 1024],
   then the 2x2 convs + scorer as tiny host-built matmuls.
"""

import os
import sys

for _p in ("/opt/trn_rl_repo",):
    if _p not in sys.path and os.path.isdir(_p):
        sys.path.insert(0, _p)

import ml_dtypes
import numpy as np

import concourse.bass as bass
import concourse.mybir as mybir
import concourse.tile as tile
from concourse.bass_utils import run_bass_kernel_spmd
from concourse.masks import make_identity

# This container's walrus build rejects CTRL instructions (Drain) carrying
# more than 2 sync waits ("Too many sync wait commands" in setupSyncWait).
# Tile's kernel-tail drain accumulates one wait per engine/DMA-queue sem, so
# redistribute: keep one wait on the drain, move the rest onto nofuse NOPs
# that execute before the all-engine barrier. Semantics are unchanged (all
# waits still complete before the barrier / semaphore teardown).
def _patched_drain_and_barrier(self, tick_clock, wait_clock):
    nc = self.nc
    drain_inst = nc.sync.drain()
    wait_clock.add_sem_waits(
        drain_inst.ins, tile.ScopedClock({None: tick_clock.global_clock})
    )
    waits = list(drain_inst.ins.sync_info.on_wait)
    if len(waits) > 1:
        drain_inst.ins.sync_info.on_wait = waits[:1]
        for k in range(1, len(waits)):
            nop = nc.sync.nop(nofuse=True, hint="drain_wait_spill")
            if nop.ins.sync_info is None:
                nop.ins.sync_info = mybir.SyncInfo(on_wait=[], on_update=[])
            nop.ins.sync_info.on_wait = [waits[k]]
    nc.all_engine_barrier()
    assert self.sems is not None
    popped = nc._tile_sem_poison_stack.pop()
    assert popped is self._sem_poison
    nc.clear_and_free_semaphores(list(self.sems.allocated().values()))
    nc.all_engine_barrier()


tile.TileContext._drain_and_barrier = _patched_drain_and_barrier

def _spill_excess_waits(nc):
    """Walrus here rejects instructions with more than ~2 sync waits. Spill
    excess waits onto same-engine NoOps inserted just before the instruction
    (engines dispatch in program order, so waiting earlier on the same engine
    is equivalent)."""
    cnt = [0]
    for fn in nc.m.functions:
        for bb in fn.blocks:
            insts = list(bb.instructions)
            out = []
            changed = False
            for inst in insts:
                si = inst.sync_info
                waits = list(si.on_wait) if si is not None and si.on_wait else []
                max_waits = 1
                if len(waits) > max_waits:
                    changed = True
                    keep = waits[-max_waits:]
                    extra = waits[:-max_waits]
                    for j in range(0, len(extra), max_waits):
                        cnt[0] += 1
                        nop = mybir.InstNoOp(name=f"I-spillw-{cnt[0]}", ins=[], outs=[])
                        nop.engine = inst.engine
                        nop.sync_info = mybir.SyncInfo(
                            on_wait=extra[j:j + max_waits], on_update=[])
                        nop.bass_nofuse = True
                        nop.bass_priority = 0
                        nop.text_hint = "spillw"
                        nop.debug = inst.debug
                        out.append(nop)
                    si.on_wait = keep
                out.append(inst)
            if changed:
                bb.instructions = out

B, NW, L, E, H, V = 8192, 4, 16, 128, 128, 64
NCORES = 8
PER = B // NCORES          # 1024 examples per core
NWORD = PER * NW           # 4096 words per core
NEC = PER // 128           # 8 example-chunks of 128
BLK = 512                  # words per PSUM chunk
NG = NWORD // 128          # 32 groups of 128 sorted words
FB = 30.0                  # freeze bias magnitude
F32 = mybir.dt.float32
BF16 = mybir.dt.bfloat16
I32 = mybir.dt.int32
AF = mybir.ActivationFunctionType
ALU = mybir.AluOpType

P6 = [(0, 1), (0, 2), (0, 3), (1, 2), (1, 3), (2, 3)]


# ----------------------------------------------------------------- host prep

def _build_consts(inp):
    emb = np.asarray(inp["emb_i"], np.float32)
    W_ih = np.asarray(inp["W_ih"], np.float32)
    W_hh = np.asarray(inp["W_hh"], np.float32)
    b = np.asarray(inp["b_ih"], np.float32) + np.asarray(inp["b_hh"], np.float32)
    # gate-column reorder (torch i,f,g,o) -> (i,f,o,g)
    gorder = np.r_[0:H, H:2 * H, 3 * H:4 * H, 2 * H:3 * H]
    G = np.zeros((V + 2, 4 * H), np.float32)
    G[:V] = (emb @ W_ih.T + b)[:, gorder]
    G[V, 0:H] = -FB            # i -> 0
    G[V, H:2 * H] = +FB        # f -> 1
    WhhT = np.ascontiguousarray(W_hh.T[:, gorder])

    w1 = np.asarray(inp["conv1_w"], np.float32)
    b1 = np.asarray(inp["conv1_b"], np.float32)
    w2 = np.asarray(inp["conv2_w"], np.float32)
    b2 = np.asarray(inp["conv2_b"], np.float32)
    ws = np.asarray(inp["scorer_w"], np.float32)
    bs = float(np.asarray(inp["scorer_b"], np.float32)[0])

    p6idx = {p: i for i, p in enumerate(P6)}
    W1eff = np.zeros((6, 36), np.float32)
    b1eff = np.zeros((36, 1), np.float32)
    for c in range(4):
        for y in range(3):
            for x in range(3):
                m = c * 9 + y * 3 + x
                b1eff[m, 0] += b1[c]
                for dy in range(2):
                    for dx in range(2):
                        a, bb = y + dy, x + dx
                        w = w1[c, 0, dy, dx]
                        if a == bb:
                            b1eff[m, 0] += w
                        else:
                            W1eff[p6idx[(min(a, bb), max(a, bb))], m] += w
    W2eff = np.zeros((36, 32), np.float32)
    b2eff = np.zeros((32, 1), np.float32)
    for c2 in range(8):
        for y in range(2):
            for x in range(2):
                m = c2 * 4 + y * 2 + x
                b2eff[m, 0] = b2[c2]
                for c1 in range(4):
                    for dy in range(2):
                        for dx in range(2):
                            W2eff[c1 * 9 + (y + dy) * 3 + (x + dx), m] += w2[c2, c1, dy, dx]
    Wsc = ws[0].astype(np.float32).reshape(32, 1)
    return dict(G65=G, WhhT=WhhT, W1eff=W1eff, b1eff=b1eff,
                W2eff=W2eff, b2eff=b2eff, Wsc=Wsc, bsc=bs)


def _core_prep(word_ids_c, lengths_c):
    wid = np.asarray(word_ids_c).reshape(NWORD, L)
    lens = np.asarray(lengths_c).reshape(NWORD)
    perm = np.argsort(-lens, kind="stable")
    wid_s = wid[perm]
    lens_s = lens[perm]
    Nt = (np.arange(L)[:, None] < lens_s[None, :]).sum(1)  # alive count per step
    # scatter destination row (example-grouped layout) per sorted position
    e = perm // NW
    i = perm % NW
    dest = (i * PER + e).astype(np.int32)          # [NWORD]
    idx = np.ascontiguousarray(dest.reshape(NG, 128).T)  # [128, NG]
    return wid_s, lens_s, Nt, idx


def _build_onehot(wid_s, lens_s, W, off, tot):
    oh = np.zeros((V + 2, tot), np.float32)
    for t in range(L):
        n = int(W[t])
        if n == 0:
            continue
        ch = np.where(lens_s[:n] > t, wid_s[:n, t], V)
        oh[ch, off[t] + np.arange(n)] = 1.0
    return oh


# -------------------------------------------------------------- bass program

def _schedule(W):
    """W: per-step widths. Returns (off, tot, groups_by_t)."""
    off = np.zeros(L, np.int64)
    for t in range(1, L):
        off[t] = off[t - 1] + W[t - 1]
    tot = int(off[-1] + W[-1])
    # group g (cols [128g,128g+128)) finalizes after the last step with W > 128g
    groups_by_t = {t: [] for t in range(L)}
    for g in range(NG):
        fg = max(t for t in range(L) if W[t] > 128 * g)
        groups_by_t[fg].append(g)
    return off, tot, groups_by_t


def _build_program(W):
    W = list(W) + [0]
    off, tot, groups_by_t = _schedule(W[:L])

    nc = bass.Bass()
    oh_in = nc.dram_tensor("oh", [V + 2, tot], BF16, kind="ExternalInput")
    idx_in = nc.dram_tensor("idx", [128, NG], I32, kind="ExternalInput")
    g65_in = nc.dram_tensor("g65", [V + 2, 4 * H], BF16, kind="ExternalInput")
    whht_in = nc.dram_tensor("whht", [H, 4 * H], BF16, kind="ExternalInput")
    w1_in = nc.dram_tensor("w1", [6, 36], BF16, kind="ExternalInput")
    b1_in = nc.dram_tensor("b1", [36, 1], F32, kind="ExternalInput")
    w2_in = nc.dram_tensor("w2", [36, 32], BF16, kind="ExternalInput")
    b2_in = nc.dram_tensor("b2", [32, 1], F32, kind="ExternalInput")
    wsc_in = nc.dram_tensor("wsc", [32, 1], BF16, kind="ExternalInput")
    bsc_in = nc.dram_tensor("bsc", [1, 1], F32, kind="ExternalInput")
    out_d = nc.dram_tensor("out", [1, PER], F32, kind="ExternalOutput")
    e_dram = nc.dram_tensor("escratch", [NWORD, H], BF16)

    with tile.TileContext(nc) as tc:
        with (
            tc.tile_pool(name="const", bufs=1) as cpool,
            tc.tile_pool(name="state", bufs=1) as spool,
        ):
            g65_sb = cpool.tile([V + 2, 4 * H], BF16, tag="g65", name="g65")
            whht_sb = cpool.tile([H, 4 * H], BF16, tag="whht", name="whht")
            idx_sb = cpool.tile([128, NG], I32, tag="idx", name="idx")
            w1_sb = cpool.tile([6, 36], BF16, tag="w1", name="w1")
            b1_sb = cpool.tile([36, 1], F32, tag="b1", name="b1")
            w2_sb = cpool.tile([36, 32], BF16, tag="w2", name="w2")
            b2_sb = cpool.tile([32, 1], F32, tag="b2", name="b2")
            wsc_sb = cpool.tile([32, 1], BF16, tag="wsc", name="wsc")
            bsc_sb = cpool.tile([1, 1], F32, tag="bsc", name="bsc")
            ident = cpool.tile([128, 128], F32, tag="ident", name="ident")
            for sb, dr in ((g65_sb, g65_in), (whht_sb, whht_in)):
                nc.sync.dma_start(sb[:], dr[:])
            make_identity(nc, ident[:])

            NBLK = (max(W[:L]) + BLK - 1) // BLK
            c_blk = [spool.tile([H, BLK], BF16, tag=f"c{k}", name=f"c{k}")
                     for k in range(NBLK)]
            h_blk = [spool.tile([H, BLK], BF16, tag=f"h{k}", name=f"h{k}")
                     for k in range(NBLK)]

            # warm the PE clock gate while the first DMAs land
            with tc.tile_pool(name="warm", bufs=1, space="PSUM") as wpsum:
                wp = wpsum.tile([128, 128], F32, tag="wp", name="wp")
                for _ in range(20):
                    nc.tensor.transpose(wp[:], ident[:], ident[:])

            # ------------------------------------------------ LSTM main loop
            with (
                tc.tile_pool(name="oh", bufs=2) as ohpool,
                tc.tile_pool(name="gates", bufs=2, space="PSUM") as gpsum,
                tc.tile_pool(name="act", bufs=3) as apool,
                tc.tile_pool(name="scat", bufs=2) as scpool,
            ):
                oh_tiles = {}
                def load_oh(t, split=False):
                    if t >= L or W[t] == 0 or t in oh_tiles:
                        return
                    sb = ohpool.tile([V + 2, NWORD], BF16, tag="oh", name="oh")
                    o0 = int(off[t])
                    if split:
                        cut = min(2 * BLK, W[t])
                        nc.sync.dma_start(sb[:, :cut], oh_in[:, o0:o0 + cut])
                        if W[t] > cut:
                            nc.sync.dma_start(sb[:, cut:W[t]],
                                              oh_in[:, o0 + cut:o0 + W[t]])
                    else:
                        nc.sync.dma_start(sb[:, :W[t]], oh_in[:, o0:o0 + W[t]])
                    oh_tiles[t] = sb

                load_oh(0, split=True)
                # tail-only constants: issue after the hot-path DMAs
                for sb, dr in ((idx_sb, idx_in), (w1_sb, w1_in), (b1_sb, b1_in),
                               (w2_sb, w2_in), (b2_sb, b2_in), (wsc_sb, wsc_in),
                               (bsc_sb, bsc_in)):
                    nc.sync.dma_start(sb[:], dr[:])
                for t in range(L):
                    Wt = W[t]
                    if Wt == 0:
                        continue
                    ct = (Wt + BLK - 1) // BLK
                    oh_sb = oh_tiles.pop(t)
                    for k in range(ct):
                        w = min(BLK, Wt - BLK * k)
                        wn = max(0, min(W[t + 1] - BLK * k, w))  # next-alive prefix
                        P = gpsum.tile([128, 4 * BLK], F32, tag="gates", name="gates")
                        # sigma gates (i,f,o) first so the wide sigmoid can
                        # start before the g matmuls finish
                        for m in (0, 1, 2):
                            nc.tensor.matmul(
                                P[:, BLK * m:BLK * m + w],
                                lhsT=g65_sb[:, H * m:H * (m + 1)],
                                rhs=oh_sb[:, BLK * k:BLK * k + w],
                                start=True, stop=(t == 0))
                        if t > 0:
                            for m in (0, 1, 2):
                                nc.tensor.matmul(
                                    P[:, BLK * m:BLK * m + w],
                                    lhsT=whht_sb[:, H * m:H * (m + 1)],
                                    rhs=h_blk[k][:, :w],
                                    start=False, stop=True)
                        nc.tensor.matmul(
                            P[:, 3 * BLK:3 * BLK + w],
                            lhsT=g65_sb[:, 3 * H:4 * H],
                            rhs=oh_sb[:, BLK * k:BLK * k + w],
                            start=True, stop=(t == 0))
                        if t > 0:
                            nc.tensor.matmul(
                                P[:, 3 * BLK:3 * BLK + w],
                                lhsT=whht_sb[:, 3 * H:4 * H],
                                rhs=h_blk[k][:, :w],
                                start=False, stop=True)
                        if k == 0:
                            load_oh(t + 1)  # prefetch ahead of freeze-group DMAs
                        ifo = apool.tile([128, 3 * BLK], BF16, tag="ifo", name="ifo")
                        nc.scalar.activation(ifo[:, :2 * BLK + w], P[:, :2 * BLK + w],
                                             AF.Sigmoid)
                        gt = apool.tile([128, BLK], BF16, tag="g", name="g")
                        nc.scalar.activation(gt[:, :w], P[:, 3 * BLK:3 * BLK + w],
                                             AF.Tanh)
                        if t == 0:
                            nc.vector.tensor_mul(c_blk[k][:, :w],
                                                 ifo[:, :w], gt[:, :w])
                        else:
                            u = apool.tile([128, BLK], BF16, tag="u", name="u")
                            nc.vector.tensor_mul(u[:, :w], ifo[:, :w], gt[:, :w])
                            nc.vector.tensor_mul(c_blk[k][:, :w],
                                                 ifo[:, BLK:BLK + w],
                                                 c_blk[k][:, :w])
                            nc.vector.tensor_add(c_blk[k][:, :w],
                                                 c_blk[k][:, :w], u[:, :w])
                        if wn > 0:
                            tct = apool.tile([128, BLK], BF16, tag="tc", name="tc")
                            nc.scalar.activation(tct[:, :wn], c_blk[k][:, :wn],
                                                 AF.Tanh)
                            nc.vector.tensor_mul(h_blk[k][:, :wn],
                                                 ifo[:, 2 * BLK:2 * BLK + wn],
                                                 tct[:, :wn])
                    # finalize groups whose last alive step was t: cast ->
                    # xbar transpose -> indirect scatter (example layout)
                    for g in groups_by_t[t]:
                        blk, rel = g // 4, (g % 4) * 128
                        tb = scpool.tile([128, 128], BF16, tag="tb", name="tb")
                        nc.sync.dma_start_transpose(
                            tb[:], c_blk[blk][:, rel:rel + 128])
                        nc.gpsimd.indirect_dma_start(
                            out=e_dram[:],
                            out_offset=bass.IndirectOffsetOnAxis(
                                ap=idx_sb[:, g:g + 1], axis=0),
                            in_=tb[:],
                            in_offset=None,
                        )

            # ------------------------------------------------------- tail
            with (
                tc.tile_pool(name="big", bufs=1) as big,
                tc.tile_pool(name="tpsum", bufs=2, space="PSUM") as tpsum,
                tc.tile_pool(name="cpsum", bufs=1, space="PSUM") as cpsum,
                tc.tile_pool(name="small", bufs=1) as small,
                tc.tile_pool(name="scr", bufs=2) as scrp,
            ):
                A = big.tile([128, NWORD], BF16, tag="A", name="A")
                NB4 = NW * NEC // 4
                for q in range(4):
                    nc.sync.dma_start(
                        A[:, q * NB4 * H:(q + 1) * NB4 * H].rearrange(
                            "p (b h) -> p b h", b=NB4),
                        e_dram[q * NB4 * 128:(q + 1) * NB4 * 128, :].rearrange(
                            "(b p) h -> p b h", p=128))
                # norms on the (otherwise idle) Scalar engine: per word-group
                # Square with free-dim accumulate -> d[:, b]
                d_all = small.tile([128, NW * NEC], F32, tag="d", name="d")
                sqscr = scrp.tile([128, 128], F32, tag="sqscr", name="sqscr")
                for b in range(NW * NEC):
                    nc.scalar.activation(sqscr[:], A[:, b * H:(b + 1) * H],
                                         AF.Square,
                                         accum_out=d_all[:, b:b + 1])
                nc.vector.tensor_scalar_max(d_all[:], d_all[:], 1e-30)
                rn = small.tile([128, NW * NEC], F32, tag="rn", name="rn")
                nc.scalar.activation(rn[:], d_all[:], AF.Ln)
                nc.scalar.activation(rn[:], rn[:], AF.Exp, scale=-0.5)

                Dp = small.tile([128, 6 * NEC], F32, tag="Dp", name="Dp")
                for k, (i, j) in enumerate(P6):
                    scr = scrp.tile([128, NEC * 128], BF16, tag="scr", name="scr")
                    nc.vector.tensor_mul(
                        scr[:], A[:, i * PER:(i + 1) * PER],
                        A[:, j * PER:(j + 1) * PER])
                    nc.vector.tensor_reduce(
                        Dp[:, k * NEC:(k + 1) * NEC],
                        scr[:].rearrange("p (e h) -> p e h", e=NEC),
                        axis=mybir.AxisListType.X, op=ALU.add)
                for k, (i, j) in enumerate(P6):
                    nc.vector.tensor_mul(Dp[:, k * NEC:(k + 1) * NEC],
                                         Dp[:, k * NEC:(k + 1) * NEC],
                                         rn[:, i * NEC:(i + 1) * NEC])
                    nc.vector.tensor_mul(Dp[:, k * NEC:(k + 1) * NEC],
                                         Dp[:, k * NEC:(k + 1) * NEC],
                                         rn[:, j * NEC:(j + 1) * NEC])

                cos6 = small.tile([6, PER], BF16, tag="cos6", name="cos6")
                for ec in range(NEC):
                    pt = tpsum.tile([128, 128], F32, tag="tp", name="tp")
                    dview = bass.AP(Dp.tensor, Dp.offset + ec,
                                    [Dp.ap[0], [NEC, 6]])
                    nc.tensor.transpose(pt[:6, :], dview, ident[:])
                    nc.vector.tensor_copy(cos6[:, ec * 128:(ec + 1) * 128], pt[:6, :])

                r1 = small.tile([36, PER], BF16, tag="r1", name="r1")
                r2 = small.tile([32, PER], BF16, tag="r2", name="r2")
                o_sb = small.tile([1, PER], F32, tag="o", name="o")
                p1 = cpsum.tile([36, PER], F32, tag="cp1", name="cp1")
                p2 = cpsum.tile([32, PER], F32, tag="cp2", name="cp2")
                p3 = cpsum.tile([1, PER], F32, tag="cp3", name="cp3")
                for half in range(2):
                    sl = slice(half * 512, (half + 1) * 512)
                    nc.tensor.matmul(p1[:, sl], lhsT=w1_sb[:], rhs=cos6[:, sl],
                                     start=True, stop=True)
                    nc.scalar.activation(r1[:, sl], p1[:, sl], AF.Relu,
                                         bias=b1_sb[:, 0:1])
                    nc.tensor.matmul(p2[:, sl], lhsT=w2_sb[:], rhs=r1[:, sl],
                                     start=True, stop=True)
                    nc.scalar.activation(r2[:, sl], p2[:, sl], AF.Relu,
                                         bias=b2_sb[:, 0:1])
                    nc.tensor.matmul(p3[:, sl], lhsT=wsc_sb[:], rhs=r2[:, sl],
                                     start=True, stop=True)
                    nc.scalar.activation(o_sb[:, sl], p3[:, sl], AF.Sigmoid,
                                         bias=bsc_sb[0:1, 0:1])
                nc.sync.dma_start(out_d[:], o_sb[:])

    return nc


_prog_cache = {}


def _get_program(W):
    key = tuple(int(x) for x in W)
    if key not in _prog_cache:
        _prog_cache[key] = _build_program(key)
    return _prog_cache[key]


def _run(inputs, trace=False):
    consts = _build_consts(inputs)
    word_ids = np.asarray(inputs["word_ids"])
    lengths = np.asarray(inputs["lengths"])

    preps = []
    for c in range(NCORES):
        sl = slice(c * PER, (c + 1) * PER)
        preps.append(_core_prep(word_ids[sl], lengths[sl]))
    Nt_max = np.stack([p[2] for p in preps]).max(0)
    W = tuple(int(min(NWORD, -(-int(n) // 16) * 16)) for n in Nt_max)
    off, tot, _ = _schedule(list(W))

    g65_bf = consts["G65"].astype(ml_dtypes.bfloat16)
    whht_bf = consts["WhhT"].astype(ml_dtypes.bfloat16)
    in_maps = []
    for c in range(NCORES):
        wid_s, lens_s, _, idx = preps[c]
        in_maps.append({
            "oh": _build_onehot(wid_s, lens_s, W, off, tot).astype(ml_dtypes.bfloat16),
            "idx": idx,
            "g65": g65_bf, "whht": whht_bf,
            "w1": consts["W1eff"].astype(ml_dtypes.bfloat16), "b1": consts["b1eff"],
            "w2": consts["W2eff"].astype(ml_dtypes.bfloat16), "b2": consts["b2eff"],
            "wsc": consts["Wsc"].astype(ml_dtypes.bfloat16),
            "bsc": np.full((1, 1), consts["bsc"], np.float32),
        })

    nc = _get_program(W)
    _spill_excess_waits(nc)  # idempotent; HW-compile only
    res = run_bass_kernel_spmd(nc, in_maps, list(range(NCORES)), trace=trace)
    out = np.concatenate([np.asarray(r["out"]).reshape(PER) for r in res.results])
    return out.reshape(B, 1).astype(np.float32), res.exec_time_ns


def kernel(**inputs):
    return _run(inputs)[0]
